# revision 1
# baseline (speedup 1.0000x reference)
"""Trainium2 Bass kernel for a YOLO-style detection loss.

Sharding: data-parallel over batch — 8 NeuronCores, 4 batches/core.
Per-core partial sums land in a [128, 16] tile; the host sums the
relevant slices of the 8 tiles and assembles the 4 scalar losses
(this host gather replaces the all-reduce of 4 scalars).

Key observation: the loss only touches pred densely through the
objectness channel (BCE vs 0 over every cell).  The class BCE term
needs the 80 class logits only at the assigned cells, and the box term
needs channels 0..3 there.  Device work:

1. OBJ stream: softplus over channel 4 of every cell (one [128, 263]
   tile), per-scale sums via DVE column reduces; the positive-cell
   correction (BCE(x,1)-BCE(x,0) = -x) comes from the gathered rows.
2. One 85-float channels-last row gather per target (indirect DMA;
   contiguous rows of a host-transposed [cells, 85] copy; 128 rows per
   call, 3 calls for up to 384 targets/core), then ~25 small DVE/ACT
   ops: box decode + l1, per-scale positive sums, class softplus sums,
   and the target-class logit correction as a one-hot dot product.

softplus(x) = ln(exp(x) + 1); Exp/Ln/Abs are pinned to the single ACT
table that holds all three (natural_log_exp_and_others) to avoid
per-instruction table reloads.  Sigmoid = 1/(1+exp(-x)) via DVE
reciprocal.  tensor_tensor_reduce is broken on this HW build, so
reductions use multiply + tensor_reduce.
"""

import numpy as np

from concourse import bass, bacc, mybir
from concourse import bass_utils
from concourse.tile import TileContext

F32 = mybir.dt.float32
I32 = mybir.dt.int32

NUM_CLASSES = 80
STAL_GAMMA = np.float32(2.0)
BATCH = 32
NCORES = 8
BPC = BATCH // NCORES          # batches per core
CH = 5 + NUM_CLASSES
HW = (80 * 80, 40 * 40, 20 * 20)
WS = (80, 40, 20)
NCELL = BPC * (HW[0] + HW[1] + HW[2])       # 33600 cells per core
COFF = (0, BPC * HW[0], BPC * (HW[0] + HW[1]))  # per-scale cell offsets
# OBJ stream: per-scale column blocks, scale 2 padded to 128*13
OBJ_COLS = (HW[0] * BPC // 128, HW[1] * BPC // 128, 1664 // 128)  # 200,50,13
NOBJ = HW[0] * BPC + HW[1] * BPC + 1664     # 33664 (64 pad cells of -100)
GROUPS = 3                                  # gather calls (128 targets each)
TPAD = 128 * GROUPS                         # 384; mean load is ~256/core
# meta column layout (GROUPS target-columns per quantity, interleaved)
MC_ADD = 0                                  # (gx, gy)          6 cols
MC_MUL = 6                                  # 1/w x4           12 cols
MC_SUB = 18                                 # (cx, cy, bw, bh) 12 cols
MC_SWM = 30                                 # small_weight/4    3 cols
MC_D0 = 33                                  # obj dedup flags   9 cols
MC_VLD = 42                                 # real-target flag  3 cols
MC_GI = 45                                  # gather row offsets (i32 bits)
MC_OH = 48                                  # class one-hot   240 cols
NMETA = MC_OH + GROUPS * NUM_CLASSES        # 288
# output partial tile column layout
OC_WSP = 0      # class softplus-sum term
OC_OBJ = 1      # 3 cols: per-scale objectness softplus sums
OC_BOX = 4
OC_POS = 5      # 3 cols
OC_CORR = 8
NOUT = 16

_NC_CACHE = None


def _ap(handle_ap, off, dims):
    return bass.AP(handle_ap.tensor, off, [list(d) for d in dims])


def _single_act_table(arch):
    """All of Exp/Ln/Abs live in natural_log_exp_and_others; hide them
    from the other tables so every activation uses one table (one load
    instead of a reload on each Exp<->Ln transition)."""
    tabs = _ORIG_TABLES(arch)
    need = {mybir.ActivationFunctionType.Exp,
            mybir.ActivationFunctionType.Ln}
    out = {}
    for name, fns in tabs.items():
        out[name] = fns if name == "natural_log_exp_and_others" \
            else fns - need
    return out


_ORIG_TABLES = bacc.get_activation_tables


def _build_nc():
    nc = bacc.Bacc("TRN2", target_bir_lowering=False, debug=False)
    fall_t = nc.dram_tensor("FALL", [NCELL * CH], F32, kind="ExternalInput")
    obj_t = nc.dram_tensor("OBJ", [128, sum(OBJ_COLS)], F32,
                           kind="ExternalInput")
    mt_t = nc.dram_tensor("MT", [128, NMETA], F32, kind="ExternalInput")
    out_t = nc.dram_tensor("OUT", [128, NOUT], F32, kind="ExternalOutput")

    EXP = mybir.ActivationFunctionType.Exp
    LN = mybir.ActivationFunctionType.Ln
    AX = mybir.AxisListType
    NOB = sum(OBJ_COLS)
    with TileContext(nc) as tc:
        with tc.tile_pool(name="persist", bufs=1) as pp:
            part = pp.tile([128, NOUT], F32)
            mt = pp.tile([128, NMETA], F32)
            va = pp.tile([128, GROUPS * CH], F32)  # per-target 85-float rows
            vt = pp.tile([128, GROUPS * NUM_CLASSES], F32)
            l1 = pp.tile([128, GROUPS], F32)
            sc = pp.tile([128, GROUPS], F32)
            g3 = pp.tile([128, GROUPS], F32)
            ob = pp.tile([128, NOB], F32)
            # meta (with bit-packed gather offsets) on the scalar HWDGE
            # ring, objectness on the sync ring - they run in parallel
            nc.scalar.dma_start(out=mt[:], in_=mt_t.ap())
            gi = mt[:, MC_GI:MC_GI + GROUPS].bitcast(I32)
            # one 85-float row per target; 128 rows (one per partition)
            # per call; target t sits at (p, j) = (t % 128, t // 128)
            for j in range(GROUPS):
                nc.gpsimd.indirect_dma_start(
                    out=va[:, CH * j:CH * j + CH], out_offset=None,
                    in_=_ap(fall_t.ap(), 0, [[1, NCELL * CH], [1, 1]]),
                    in_offset=bass.IndirectOffsetOnAxis(ap=gi[:, j:j + 1],
                                                        axis=0))

            nc.sync.dma_start(out=ob[:], in_=obj_t.ap())
            nc.vector.memset(part[:], 0.0)

            # ---- dense objectness stream ----
            nc.scalar.activation(ob[:], ob[:], EXP)
            nc.scalar.activation(ob[:], ob[:], LN, bias=1.0)
            ocol = 0
            for s in range(3):
                w = OBJ_COLS[s]
                nc.vector.reduce_sum(part[:, OC_OBJ + s:OC_OBJ + s + 1],
                                     ob[:, ocol:ocol + w], axis=AX.X)
                ocol += w

            # ---- per-target math ----
            va3 = va[:].rearrange("p (j c) -> p j c", c=CH)
            vt3 = vt[:].rearrange("p (j c) -> p j c", c=NUM_CLASSES)
            mt3 = lambda lo, w: mt[:, lo:lo + GROUPS * w].rearrange(
                "p (j c) -> p j c", c=w)
            # box decode: ch0,1 -> sigmoid = 1/(1+exp(-x)) ; ch2,3 ->
            # exp(min(x,4)); one shared EXP pass over ch0..3
            nc.vector.tensor_scalar_mul(va3[:, :, 0:2], va3[:, :, 0:2], -1.0)
            nc.vector.tensor_scalar_min(va3[:, :, 2:4], va3[:, :, 2:4], 4.0)
            nc.scalar.activation(va3[:, :, 0:4], va3[:, :, 0:4], EXP)
            nc.vector.tensor_scalar_add(va3[:, :, 0:2], va3[:, :, 0:2], 1.0)
            nc.vector.reciprocal(va3[:, :, 0:2], va3[:, :, 0:2])
            nc.vector.tensor_mul(va3[:, :, 0:4], va3[:, :, 0:4], mt3(MC_MUL, 4))
            nc.vector.tensor_sub(va3[:, :, 0:4], va3[:, :, 0:4], mt3(MC_SUB, 4))
            nc.vector.reduce_sum(l1[:], va3[:, :, 0:4], axis=AX.X,
                                 apply_absolute_value=True)
            nc.vector.tensor_mul(l1[:], l1[:], mt[:, MC_SWM:MC_SWM + GROUPS])
            nc.vector.reduce_sum(part[:, OC_BOX:OC_BOX + 1], l1[:], axis=AX.X)
            # class-logit correction: one-hot dot with the raw logits
            nc.vector.tensor_mul(vt3, va3[:, :, 5:CH], mt3(MC_OH, NUM_CLASSES))
            nc.vector.reduce_sum(part[:, OC_CORR:OC_CORR + 1], vt[:],
                                 axis=AX.X)
            # objectness positive-cell correction (raw channel 4)
            for s in range(3):
                nc.vector.tensor_mul(
                    sc[:], va3[:, :, 4],
                    mt[:, MC_D0 + GROUPS * s:MC_D0 + GROUPS * s + GROUPS])
                nc.vector.reduce_sum(part[:, OC_POS + s:OC_POS + s + 1],
                                     sc[:], axis=AX.X)
            # class softplus sum over the 80 logits of each target's cell
            nc.scalar.activation(va3[:, :, 5:CH], va3[:, :, 5:CH], EXP)
            nc.scalar.activation(va3[:, :, 5:CH], va3[:, :, 5:CH], LN,
                                 bias=1.0)
            nc.vector.reduce_sum(g3[:], va3[:, :, 5:CH], axis=AX.X)
            nc.vector.tensor_mul(g3[:], g3[:], mt[:, MC_VLD:MC_VLD + GROUPS])
            nc.vector.reduce_sum(part[:, OC_WSP:OC_WSP + 1], g3[:], axis=AX.X)

            nc.sync.dma_start(out=out_t.ap(), in_=part[:])
    bacc.get_activation_tables = _single_act_table
    try:
        nc.compile()
    finally:
        bacc.get_activation_tables = _ORIG_TABLES
    return nc


def get_nc():
    global _NC_CACHE
    if _NC_CACHE is None:
        _NC_CACHE = _build_nc()
    return _NC_CACHE


def prepare_in_maps(pred0, pred1, pred2, targets):
    """Host-side sharding + layout/index preprocessing (numpy only)."""
    preds = (np.asarray(pred0, dtype=np.float32),
             np.asarray(pred1, dtype=np.float32),
             np.asarray(pred2, dtype=np.float32))
    t = np.asarray(targets, dtype=np.float32)
    n = t.shape[0]
    b = t[:, 0].astype(np.int32)
    cls = t[:, 1].astype(np.int32)
    cx, cy, bw, bh = t[:, 2], t[:, 3], t[:, 4], t[:, 5]

    area = np.maximum(bw * bh, np.float32(1e-6))
    s_idx = np.where(area <= 0.01, 0,
                     np.where(area <= 0.03, 1, 2)).astype(np.int32)
    sw = np.float32(1.0) + STAL_GAMMA * (np.float32(1.0) - np.sqrt(area))

    ws = np.array(WS, np.int32)[s_idx]
    wf = ws.astype(np.float32)
    gx = np.clip((cx * wf).astype(np.int32), 0, ws - 1)
    gy = np.clip((cy * wf).astype(np.int32), 0, ws - 1)
    hw = np.array(HW, np.int64)[s_idx]

    b_cl = np.clip(b, 0, BATCH - 1)
    core = b_cl // BPC
    bl = (b_cl % BPC).astype(np.int64)
    cell = (np.array(COFF, np.int64)[s_idx] + bl * hw
            + (gy.astype(np.int64) * ws + gx))

    valid_cls = ((cls >= 0) & (cls < NUM_CLASSES)).astype(np.float32)
    cls_c = np.clip(cls, 0, NUM_CLASSES - 1)

    # obj dedup: one representative target per (scale, batch, gy, gx) cell
    key = ((s_idx.astype(np.int64) * BATCH + b_cl) * 128 + gy) * 128 + gx
    dflag = np.zeros(n, np.float32)
    _, first = np.unique(key, return_index=True)
    dflag[first] = 1.0

    in_maps = []
    for c in range(NCORES):
        sel = np.nonzero(core == c)[0]
        if len(sel) > TPAD:
            sel = sel[:TPAD]  # graceful degradation; never expected
        m = len(sel)
        csel = cell[sel]

        # target t maps to (partition, group) = (t % 128, t // 128)
        ga = np.zeros(TPAD, np.int64)
        ga[:m] = csel * CH

        mt = np.zeros((128, NMETA), np.float32)
        mt[:, MC_GI:MC_GI + GROUPS] = np.ascontiguousarray(
            ga.astype(np.int32).reshape(GROUPS, 128).T).view(np.float32)

        def put(col, vals):
            buf = np.zeros(TPAD, np.float32)
            buf[:m] = vals
            mt[:, col:col + GROUPS] = buf.reshape(GROUPS, 128).T

        def put_il(col, width, *vals):  # channel-interleaved group
            buf = np.zeros((TPAD, width), np.float32)
            for i, v in enumerate(vals):
                buf[:m, i] = v
            mt[:, col:col + GROUPS * width] = buf.reshape(
                GROUPS, 128, width).transpose(1, 0, 2).reshape(
                128, GROUPS * width)

        invw = np.float32(1.0) / wf[sel]
        put_il(MC_MUL, 4, invw, invw, invw, invw)
        put_il(MC_SUB, 4,
               cx[sel] - gx[sel].astype(np.float32) * invw,
               cy[sel] - gy[sel].astype(np.float32) * invw,
               bw[sel], bh[sel])
        put(MC_SWM, sw[sel] * np.float32(0.25))
        for s in range(3):
            put(MC_D0 + GROUPS * s, dflag[sel] * (s_idx[sel] == s))
        put(MC_VLD, np.float32(1.0))
        oh = np.zeros((TPAD, NUM_CLASSES), np.float32)
        oh[np.arange(m), cls_c[sel]] = valid_cls[sel]
        mt[:, MC_OH:] = oh.reshape(GROUPS, 128, NUM_CLASSES).transpose(
            1, 0, 2).reshape(128, GROUPS * NUM_CLASSES)

        lo, hi = c * BPC, (c + 1) * BPC
        fall = np.empty((NCELL, CH), np.float32)
        obj = np.full((128, sum(OBJ_COLS)), np.float32(-100.0), np.float32)
        off = 0
        ocol = 0
        for s, p in enumerate(preds):
            nc_s = BPC * HW[s]
            blk = p[lo:hi].reshape(BPC, CH, HW[s])
            fall[off:off + nc_s] = np.moveaxis(blk, 1, 2).reshape(nc_s, CH)
            w = OBJ_COLS[s]
            tmp = np.full(128 * w, np.float32(-100.0), np.float32)
            tmp[:nc_s] = blk[:, 4].reshape(-1)
            obj[:, ocol:ocol + w] = tmp.reshape(128, w)
            off += nc_s
            ocol += w

        in_maps.append({
            "FALL": fall.reshape(-1),
            "OBJ": obj,
            "MT": mt,
        })
    return in_maps, n


def finalize(results, n):
    """Combine per-core [128, NOUT] partial tiles into the 4 losses."""
    ps = np.stack([np.asarray(r["OUT"], np.float64) for r in results])
    cls_sp = ps[:, :, OC_WSP].sum()
    obj_sp = [ps[:, :, OC_OBJ + s].sum() for s in range(3)]
    box = ps[:, :, OC_BOX].sum()
    pos = [ps[:, :, OC_POS + s].sum() for s in range(3)]
    corr = ps[:, :, OC_CORR].sum()

    norm = max(1, n)
    box_loss = box / norm
    cls_loss = (cls_sp - corr) / (NUM_CLASSES * norm)
    obj_loss = sum((obj_sp[s] - pos[s]) / (BATCH * HW[s]) for s in range(3))
    total = box_loss + obj_loss + cls_loss
    return np.array([total, box_loss, obj_loss, cls_loss], np.float32)


def run_on_hw(in_maps, trace=False):
    nc = get_nc()
    return bass_utils.run_bass_kernel_spmd(
        nc, in_maps, core_ids=list(range(NCORES)), trace=trace)


def kernel(pred0, pred1, pred2, targets, **_unused):
    in_maps, n = prepare_in_maps(pred0, pred1, pred2, targets)
    res = run_on_hw(in_maps)
    return finalize(res.results, n)



# revision 3
# speedup vs baseline: 1.4080x; 1.4080x over previous
"""Trainium2 Bass kernel for a YOLO-style detection loss.

Sharding: data-parallel over batch — 8 NeuronCores, 4 batches/core.
Per-core partial sums land in a [128, 7] tile; the host sums the 8
tiles and assembles the 4 scalar losses (replacing the all-reduce of
4 scalars).

The loss touches pred densely only through the objectness channel
(BCE vs 0 over every cell); the class/box terms need the 85 logits at
the <=2048 assigned cells.  The host routes data (extracts channel 4,
gathers the 85-float rows per target, precomputes target-derived
constants: grid offsets, small_weight, dedup flags) — all pure data
movement/indexing, as in the indirect-DMA version, but the gather now
happens host-side so the device never pays the serialized GpSimd
indirect-DMA issue + latency chain.  All loss arithmetic on pred
values runs on device:

1. OBJ stream: softplus over channel 4 of every cell (one [128, 263]
   bf16 tile): Exp pass then per-scale Ln(1+x) passes whose accum_out
   gives the per-scale column sums directly.
2. TGT stream: one [128, 276] bf16 tile holding 3 groups x 85 gathered
   logits + per-target constants.  Box decode uses Exp(scale=-1) +
   add/reciprocal for sigmoid, Exp(min(x,4)) for wh; the weighted sums
   come out of scalar_tensor_tensor accum_out.  Class softplus sum
   falls out of the Ln pass's accum_out; the target-class logit
   correction is a host-gathered column the device just sums.

softplus(x) = ln(exp(x) + 1); Exp/Ln are pinned to the single ACT
table that holds both (natural_log_exp_and_others) to avoid
per-instruction table reloads.  bf16 tiles: all accumulations land in
fp32 (accum_out / reduce dst); tolerance is 2e-2, bf16 input rounding
contributes ~1e-4.
"""

import numpy as np
import ml_dtypes

from concourse import bass, bacc, mybir
from concourse import bass_utils
from concourse.tile import TileContext

F32 = mybir.dt.float32
BF16 = mybir.dt.bfloat16
BF16_NP = ml_dtypes.bfloat16

NUM_CLASSES = 80
STAL_GAMMA = np.float32(2.0)
BATCH = 32
NCORES = 8
BPC = BATCH // NCORES          # batches per core
CH = 5 + NUM_CLASSES
HW = (80 * 80, 40 * 40, 20 * 20)
WS = (80, 40, 20)
# OBJ stream: per-scale column blocks, scale 2 padded to 128*13
OBJ_COLS = (HW[0] * BPC // 128, HW[1] * BPC // 128, 1664 // 128)  # 200,50,13
GROUPS = 3                                  # 128 targets each
TPAD = 128 * GROUPS                         # 384; mean load is ~256/core
PAD_VAL = np.float32(-30.0)                 # neutral logit for padding rows
# TGT tile column layout (bf16), GROUPS-interleaved like the VA rows
TC_VA = 0                                   # 3 x 85 gathered logits
TC_SUB = GROUPS * CH                        # 255: w*cx-gx etc, 3 x 4
TC_SWM = TC_SUB + GROUPS * 4                # 267: sw/4/w              3
TC_WOB = TC_SWM + GROUPS                    # 270: dedup/(B*HW_s)      3
TC_COR = TC_WOB + GROUPS                    # 273: target-class logit  3
NTGT = TC_COR + GROUPS                      # 276
# output partial tile column layout
OC_WSP = 0      # class softplus-sum term
OC_OBJ = 1      # 3 cols: per-scale objectness softplus sums
OC_BOX = 4
OC_POS = 5      # objectness positive-cell correction (pre-scaled)
OC_CORR = 6
NOUT = 7

_NC_CACHE = None


def _single_act_table(arch):
    """All of Exp/Ln live in natural_log_exp_and_others; hide them from
    the other tables so every activation uses one table (one load
    instead of a reload on each Exp<->Ln transition)."""
    tabs = _ORIG_TABLES(arch)
    need = {mybir.ActivationFunctionType.Exp,
            mybir.ActivationFunctionType.Ln}
    out = {}
    for name, fns in tabs.items():
        out[name] = fns if name == "natural_log_exp_and_others" \
            else fns - need
    return out


_ORIG_TABLES = bacc.get_activation_tables


def _build_nc():
    nc = bacc.Bacc("TRN2", target_bir_lowering=False, debug=False)
    obj_t = nc.dram_tensor("OBJ", [128, sum(OBJ_COLS)], BF16,
                           kind="ExternalInput")
    tgt_t = nc.dram_tensor("TGT", [128, NTGT], BF16, kind="ExternalInput")
    out_t = nc.dram_tensor("OUT", [128, NOUT], F32, kind="ExternalOutput")

    EXP = mybir.ActivationFunctionType.Exp
    LN = mybir.ActivationFunctionType.Ln
    AX = mybir.AxisListType
    ALU = mybir.AluOpType
    NOB = sum(OBJ_COLS)
    with nc.allow_low_precision("bf16 validated: tolerance 2e-2, "
                                "bf16 rounding contributes ~1e-4"), \
            TileContext(nc) as tc:
        with tc.tile_pool(name="persist", bufs=1) as pp:
            part = pp.tile([128, NOUT], F32)
            tg = pp.tile([128, NTGT], BF16)
            ob = pp.tile([128, NOB], BF16)
            l1 = pp.tile([128, GROUPS], BF16)
            g3 = pp.tile([128, GROUPS], BF16)
            sc = pp.tile([128, GROUPS], BF16)

            # TGT on the sync HWDGE ring, OBJ on the scalar ring — they
            # start and transfer in parallel; OUT reuses the warm sync
            # ring at the end.
            nc.sync.dma_start(out=tg[:], in_=tgt_t.ap())
            nc.scalar.dma_start(out=ob[:], in_=obj_t.ap())

            v3 = tg[:, :TC_SUB].rearrange("p (j c) -> p j c", c=CH)
            sub3 = tg[:, TC_SUB:TC_SWM].rearrange("p (j c) -> p j c", c=4)

            # ---- dense objectness stream (ACT engine) ----
            nc.scalar.activation(ob[:], ob[:], EXP)
            # box decode transcendentals interleave with the obj Ln
            # passes so the DVE chain can start early
            nc.vector.tensor_scalar_min(v3[:, :, 2:4], v3[:, :, 2:4], 4.0)
            nc.scalar.activation(v3[:, :, 0:2], v3[:, :, 0:2], EXP,
                                 scale=-1.0)
            nc.scalar.activation(v3[:, :, 2:4], v3[:, :, 2:4], EXP)
            ocol = 0
            for s in range(3):
                w = OBJ_COLS[s]
                nc.scalar.activation(
                    ob[:, ocol:ocol + w], ob[:, ocol:ocol + w], LN, bias=1.0,
                    accum_out=part[:, OC_OBJ + s:OC_OBJ + s + 1])
                ocol += w

            # ---- per-target math ----
            # sigmoid = 1/(1+exp(-x)) via DVE reciprocal
            nc.vector.tensor_scalar_add(v3[:, :, 0:2], v3[:, :, 0:2], 1.0)
            nc.vector.reciprocal(v3[:, :, 0:2], v3[:, :, 0:2])
            # objectness positive-cell correction (raw channel 4),
            # per-scale 1/(B*HW_s) prefolded into the WOB column
            nc.vector.scalar_tensor_tensor(
                sc[:], v3[:, :, 4], 0.0, tg[:, TC_WOB:TC_WOB + GROUPS],
                op0=ALU.bypass, op1=ALU.mult,
                accum_out=part[:, OC_POS:OC_POS + 1])
            # l1 in grid units: |dec - (w*tgt - g)|; 1/w folded into SWM
            nc.vector.tensor_sub(v3[:, :, 0:4], v3[:, :, 0:4], sub3)
            nc.vector.reduce_sum(l1[:], v3[:, :, 0:4], axis=AX.X,
                                 apply_absolute_value=True)
            nc.vector.scalar_tensor_tensor(
                g3[:], l1[:], 0.0, tg[:, TC_SWM:TC_SWM + GROUPS],
                op0=ALU.bypass, op1=ALU.mult,
                accum_out=part[:, OC_BOX:OC_BOX + 1])
            # class-logit correction: host-gathered column, just sum it
            nc.vector.reduce_sum(part[:, OC_CORR:OC_CORR + 1],
                                 tg[:, TC_COR:TC_COR + GROUPS], axis=AX.X)
            # class softplus sum over the 80 logits of each target's cell
            nc.scalar.activation(v3[:, :, 5:CH], v3[:, :, 5:CH], EXP)
            nc.scalar.activation(v3[:, :, 5:CH], v3[:, :, 5:CH], LN,
                                 bias=1.0,
                                 accum_out=part[:, OC_WSP:OC_WSP + 1])

            nc.sync.dma_start(out=out_t.ap(), in_=part[:])
    bacc.get_activation_tables = _single_act_table
    try:
        nc.compile()
    finally:
        bacc.get_activation_tables = _ORIG_TABLES
    return nc


def get_nc():
    global _NC_CACHE
    if _NC_CACHE is None:
        _NC_CACHE = _build_nc()
    return _NC_CACHE


def prepare_in_maps(pred0, pred1, pred2, targets):
    """Host-side sharding + layout/index preprocessing (numpy only)."""
    preds = (np.asarray(pred0, dtype=np.float32),
             np.asarray(pred1, dtype=np.float32),
             np.asarray(pred2, dtype=np.float32))
    t = np.asarray(targets, dtype=np.float32)
    n = t.shape[0]
    b = t[:, 0].astype(np.int32)
    cls = t[:, 1].astype(np.int32)
    cx, cy, bw, bh = t[:, 2], t[:, 3], t[:, 4], t[:, 5]

    area = np.maximum(bw * bh, np.float32(1e-6))
    s_idx = np.where(area <= 0.01, 0,
                     np.where(area <= 0.03, 1, 2)).astype(np.int32)
    sw = np.float32(1.0) + STAL_GAMMA * (np.float32(1.0) - np.sqrt(area))

    ws = np.array(WS, np.int32)[s_idx]
    wf = ws.astype(np.float32)
    gx = np.clip((cx * wf).astype(np.int32), 0, ws - 1)
    gy = np.clip((cy * wf).astype(np.int32), 0, ws - 1)

    b_cl = np.clip(b, 0, BATCH - 1)
    core = b_cl // BPC

    valid_cls = ((cls >= 0) & (cls < NUM_CLASSES)).astype(np.float32)
    cls_c = np.clip(cls, 0, NUM_CLASSES - 1)

    # gather the 85-float pred row for every target (pure data movement)
    va_all = np.empty((n, CH), np.float32)
    for s in range(3):
        m = np.nonzero(s_idx == s)[0]
        if len(m):
            va_all[m] = preds[s][b_cl[m], :, gy[m], gx[m]]
    corr_all = va_all[np.arange(n), 5 + cls_c] * valid_cls

    # obj dedup: one representative target per (scale, batch, gy, gx) cell
    key = ((s_idx.astype(np.int64) * BATCH + b_cl) * 128 + gy) * 128 + gx
    dflag = np.zeros(n, np.float32)
    _, first = np.unique(key, return_index=True)
    dflag[first] = 1.0
    wobj_all = dflag / (np.float32(BATCH) * np.array(HW, np.float32)[s_idx])

    hw_denoms = np.array(HW, np.float32)
    in_maps = []
    for c in range(NCORES):
        sel = np.nonzero(core == c)[0]
        if len(sel) > TPAD:
            sel = sel[:TPAD]  # graceful degradation; never expected
        m = len(sel)

        # target t maps to (partition, group) = (t % 128, t // 128)
        def put_il(width, vals):  # [m, width] -> [128, GROUPS*width]
            buf = np.zeros((TPAD, width), np.float32)
            buf[:m] = vals
            return buf.reshape(GROUPS, 128, width).transpose(1, 0, 2).reshape(
                128, GROUPS * width)

        tgt = np.empty((128, NTGT), np.float32)
        va = np.full((TPAD, CH), PAD_VAL, np.float32)
        va[:m] = va_all[sel]
        tgt[:, TC_VA:TC_SUB] = va.reshape(GROUPS, 128, CH).transpose(
            1, 0, 2).reshape(128, GROUPS * CH)
        invw = np.float32(1.0) / wf[sel]
        tgt[:, TC_SUB:TC_SWM] = put_il(4, np.stack([
            cx[sel] * wf[sel] - gx[sel],
            cy[sel] * wf[sel] - gy[sel],
            bw[sel] * wf[sel],
            bh[sel] * wf[sel]], axis=1))
        tgt[:, TC_SWM:TC_WOB] = put_il(1, (sw[sel] * np.float32(0.25)
                                           * invw)[:, None])
        tgt[:, TC_WOB:TC_COR] = put_il(1, wobj_all[sel][:, None])
        tgt[:, TC_COR:NTGT] = put_il(1, corr_all[sel][:, None])

        lo, hi = c * BPC, (c + 1) * BPC
        obj = np.full((128, sum(OBJ_COLS)), np.float32(-100.0), np.float32)
        ocol = 0
        for s, p in enumerate(preds):
            nc_s = BPC * HW[s]
            w = OBJ_COLS[s]
            tmp = np.full(128 * w, np.float32(-100.0), np.float32)
            tmp[:nc_s] = p[lo:hi, 4].reshape(-1)
            obj[:, ocol:ocol + w] = tmp.reshape(128, w)
            ocol += w

        in_maps.append({
            "OBJ": obj.astype(BF16_NP),
            "TGT": tgt.astype(BF16_NP),
        })
    return in_maps, n


def finalize(results, n):
    """Combine per-core [128, NOUT] partial tiles into the 4 losses."""
    ps = np.stack([np.asarray(r["OUT"], np.float64) for r in results])
    cls_sp = ps[:, :, OC_WSP].sum()
    obj_sp = [ps[:, :, OC_OBJ + s].sum() for s in range(3)]
    box = ps[:, :, OC_BOX].sum()
    pos = ps[:, :, OC_POS].sum()
    corr = ps[:, :, OC_CORR].sum()

    norm = max(1, n)
    box_loss = box / norm
    cls_loss = (cls_sp - corr) / (NUM_CLASSES * norm)
    obj_loss = sum(obj_sp[s] / (BATCH * HW[s]) for s in range(3)) - pos
    total = box_loss + obj_loss + cls_loss
    return np.array([total, box_loss, obj_loss, cls_loss], np.float32)


def run_on_hw(in_maps, trace=False):
    nc = get_nc()
    return bass_utils.run_bass_kernel_spmd(
        nc, in_maps, core_ids=list(range(NCORES)), trace=trace)


def kernel(pred0, pred1, pred2, targets, **_unused):
    in_maps, n = prepare_in_maps(pred0, pred1, pred2, targets)
    res = run_on_hw(in_maps)
    return finalize(res.results, n)
